# revision 1
# baseline (speedup 1.0000x reference)
"""Equivariant MHA on 8 Trainium2 NeuronCores.

Sharding: 8 cores = batch (2) x query-blocks (4 of 512). Each core holds
the full keys/values for its batch element but only its query block and
the matching Q_basis[qblock] slice, so the 134 MB Q_basis is read once
in aggregate (and each core only touches 33.5 MB of it).

Hardcoded problem shape: B=2, S=2048, D=1024, H=16, HD=64, C=8.
"""

import numpy as np

B, S, D = 2, 2048, 1024
H, HD = 16, 64
C = 8
EPS = 1e-6
NCORES = 8
QB_PER_BATCH = 4          # query blocks per batch element
QBLK = S // QB_PER_BATCH  # 512 queries per core


def _shard_fn(xq, x_b, qbasis, Wq, bq, Wk, bk, Wv, bv, q_ln_scale,
              k_ln_scale, relative_attn, Wo, bo):
    import jax.numpy as jnp
    import jax

    def ln(t, scale):
        mu = jnp.mean(t, axis=-1, keepdims=True)
        var = jnp.mean(jnp.square(t - mu), axis=-1, keepdims=True)
        return (t - mu) * jax.lax.rsqrt(var + EPS) * scale

    q = jnp.einsum('sd,dhk->shk', xq, Wq) + bq
    k = jnp.einsum('sd,dhk->shk', x_b, Wk) + bk
    v = jnp.einsum('sd,dhk->shk', x_b, Wv) + bv
    q = ln(q, q_ln_scale)
    k = ln(k, k_ln_scale)
    bias = jnp.einsum('ch,qkc->hqk', relative_attn, qbasis)
    scores = jnp.einsum('qhd,khd->hqk', q, k) / jnp.sqrt(jnp.float32(HD))
    attn = jax.nn.softmax(scores + bias, axis=-1)
    ctx = jnp.einsum('hqk,khd->qhd', attn, v)
    return jnp.einsum('qhd,hdo->qo', ctx, Wo) + bo


def kernel(**inputs):
    import jax

    x = np.asarray(inputs['x'], np.float32)
    qb = np.asarray(inputs['Q_basis'], np.float32)

    # Per-core input stacks: core i -> (b, qblk) = (i // 4, i % 4)
    xq = np.stack([x[i // QB_PER_BATCH,
                     (i % QB_PER_BATCH) * QBLK:(i % QB_PER_BATCH + 1) * QBLK]
                   for i in range(NCORES)])
    xb = np.stack([x[i // QB_PER_BATCH] for i in range(NCORES)])
    qbs = np.stack([qb[(i % QB_PER_BATCH) * QBLK:(i % QB_PER_BATCH + 1) * QBLK]
                    for i in range(NCORES)])

    rep = lambda name: np.broadcast_to(
        np.asarray(inputs[name], np.float32),
        (NCORES,) + np.asarray(inputs[name]).shape)

    f = jax.pmap(_shard_fn, devices=jax.devices()[:NCORES])
    shards = f(xq, xb, qbs, rep('Wq'), rep('bq'), rep('Wk'), rep('bk'),
               rep('Wv'), rep('bv'), rep('q_ln_scale'), rep('k_ln_scale'),
               rep('relative_attn'), rep('Wo'), rep('bo'))
    shards = np.asarray(shards)  # [8, 512, 1024]

    out = np.empty((B, S, D), np.float32)
    for i in range(NCORES):
        b, j = i // QB_PER_BATCH, i % QB_PER_BATCH
        out[b, j * QBLK:(j + 1) * QBLK] = shards[i]
    return out


# revision 2
# speedup vs baseline: 1.4760x; 1.4760x over previous
"""Equivariant MHA on 8 Trainium2 NeuronCores.

Sharding: 8 cores = batch (2) x query-blocks (4 of 512). Each core holds
the full keys/values for its batch element but only its query block and
the matching Q_basis[qblock] slice.

Transfer-size notes (axon-tunneled PJRT makes host->device bytes the
bottleneck): weights go over with in_axes=None instead of host-side
broadcast stacks, and Q_basis — the 134 MB tensor — is shipped as
bf16. The relative-position bias it produces is a ~0.06-sigma additive
perturbation on ~1-sigma attention scores, so bf16 rounding there
contributes ~1e-4 relative error to the output.

Hardcoded problem shape: B=2, S=2048, D=1024, H=16, HD=64, C=8.
"""

import numpy as np

B, S, D = 2, 2048, 1024
H, HD = 16, 64
C = 8
EPS = 1e-6
NCORES = 8
QB_PER_BATCH = 4          # query blocks per batch element
QBLK = S // QB_PER_BATCH  # 512 queries per core

_cache = {}


def _shard_fn(xq, x_b, qbasis, Wq, bq, Wk, bk, Wv, bv, q_ln_scale,
              k_ln_scale, relative_attn, Wo, bo):
    import jax.numpy as jnp
    import jax

    def ln(t, scale):
        mu = jnp.mean(t, axis=-1, keepdims=True)
        var = jnp.mean(jnp.square(t - mu), axis=-1, keepdims=True)
        return (t - mu) * jax.lax.rsqrt(var + EPS) * scale

    q = jnp.einsum('sd,dhk->shk', xq, Wq) + bq
    k = jnp.einsum('sd,dhk->shk', x_b, Wk) + bk
    v = jnp.einsum('sd,dhk->shk', x_b, Wv) + bv
    q = ln(q, q_ln_scale)
    k = ln(k, k_ln_scale)
    bias = jnp.einsum('ch,qkc->hqk', relative_attn,
                      qbasis.astype(jnp.float32))
    scores = jnp.einsum('qhd,khd->hqk', q, k) / jnp.sqrt(jnp.float32(HD))
    attn = jax.nn.softmax(scores + bias, axis=-1)
    ctx = jnp.einsum('hqk,khd->qhd', attn, v)
    return jnp.einsum('qhd,hdo->qo', ctx, Wo) + bo


def kernel(**inputs):
    import jax
    import ml_dtypes

    x = np.asarray(inputs['x'], np.float32)
    qb = np.asarray(inputs['Q_basis'], np.float32)

    # Per-core stacks: core i -> (b, qblk) = (i // 4, i % 4)
    xq = np.stack([x[i // QB_PER_BATCH,
                     (i % QB_PER_BATCH) * QBLK:(i % QB_PER_BATCH + 1) * QBLK]
                   for i in range(NCORES)])
    xb = np.stack([x[i // QB_PER_BATCH] for i in range(NCORES)])
    qbs = np.stack([qb[(i % QB_PER_BATCH) * QBLK:(i % QB_PER_BATCH + 1) * QBLK]
                    for i in range(NCORES)]).astype(ml_dtypes.bfloat16)

    if 'f' not in _cache:
        _cache['f'] = jax.pmap(
            _shard_fn,
            in_axes=(0, 0, 0) + (None,) * 11,
            devices=jax.devices()[:NCORES])
    f = _cache['f']
    w = lambda name: np.asarray(inputs[name], np.float32)
    shards = f(xq, xb, qbs, w('Wq'), w('bq'), w('Wk'), w('bk'),
               w('Wv'), w('bv'), w('q_ln_scale'), w('k_ln_scale'),
               w('relative_attn'), w('Wo'), w('bo'))
    shards = np.asarray(shards)  # [8, 512, 1024]

    out = np.empty((B, S, D), np.float32)
    for i in range(NCORES):
        b, j = i // QB_PER_BATCH, i % QB_PER_BATCH
        out[b, j * QBLK:(j + 1) * QBLK] = shards[i]
    return out


# revision 3
# speedup vs baseline: 1.9753x; 1.3383x over previous
"""Equivariant MHA on 8 Trainium2 NeuronCores.

Sharding: 8 cores = 8 distinct query blocks of 256 rows; each core
computes BOTH batch elements for its block. That makes every shipped
Q_basis byte unique (67 MB bf16 total instead of 2x), at the price of
duplicating the K/V projections per batch across cores — cheap on the
TensorEngine, and host->device transfer over the axon tunnel is the
actual bottleneck.

Numerics: Q_basis and the K/V-path x go over as bf16. The bias term is
a ~0.06-sigma additive perturbation on ~1-sigma attention scores and
K/V feed post-layernorm dot products, so bf16 rounding there stays
around 1e-3 relative on the output. The query-path x and all weights
stay fp32.

Hardcoded problem shape: B=2, S=2048, D=1024, H=16, HD=64, C=8.
"""

import numpy as np

B, S, D = 2, 2048, 1024
H, HD = 16, 64
C = 8
EPS = 1e-6
NCORES = 8
QBLK = S // NCORES  # 256 query rows per core

_cache = {}


def _shard_fn(xq, qbasis, x_full, Wq, bq, Wk, bk, Wv, bv, q_ln_scale,
              k_ln_scale, relative_attn, Wo, bo):
    import jax.numpy as jnp
    import jax

    def ln(t, scale):
        mu = jnp.mean(t, axis=-1, keepdims=True)
        var = jnp.mean(jnp.square(t - mu), axis=-1, keepdims=True)
        return (t - mu) * jax.lax.rsqrt(var + EPS) * scale

    xf = x_full.astype(jnp.float32)
    q = jnp.einsum('bsd,dhk->bshk', xq, Wq) + bq          # [2,256,H,HD]
    k = jnp.einsum('bsd,dhk->bshk', xf, Wk) + bk          # [2,S,H,HD]
    v = jnp.einsum('bsd,dhk->bshk', xf, Wv) + bv
    q = ln(q, q_ln_scale)
    k = ln(k, k_ln_scale)
    bias = jnp.einsum('ch,qkc->hqk', relative_attn,
                      qbasis.astype(jnp.float32))          # [H,256,S]
    scores = jnp.einsum('bqhd,bkhd->bhqk', q, k) / jnp.sqrt(jnp.float32(HD))
    attn = jax.nn.softmax(scores + bias[None], axis=-1)
    ctx = jnp.einsum('bhqk,bkhd->bqhd', attn, v)
    return jnp.einsum('bqhd,hdo->bqo', ctx, Wo) + bo       # [2,256,D]


def kernel(**inputs):
    import jax
    import ml_dtypes

    x = np.asarray(inputs['x'], np.float32)
    qb = np.asarray(inputs['Q_basis'], np.float32)

    xq = np.stack([x[:, i * QBLK:(i + 1) * QBLK] for i in range(NCORES)])
    qbs = np.stack([qb[i * QBLK:(i + 1) * QBLK]
                    for i in range(NCORES)]).astype(ml_dtypes.bfloat16)
    x_bf = x.astype(ml_dtypes.bfloat16)

    if 'f' not in _cache:
        _cache['f'] = jax.pmap(
            _shard_fn,
            in_axes=(0, 0) + (None,) * 12,
            devices=jax.devices()[:NCORES])
    f = _cache['f']
    w = lambda name: np.asarray(inputs[name], np.float32)
    shards = f(xq, qbs, x_bf, w('Wq'), w('bq'), w('Wk'), w('bk'),
               w('Wv'), w('bv'), w('q_ln_scale'), w('k_ln_scale'),
               w('relative_attn'), w('Wo'), w('bo'))
    shards = np.asarray(shards)  # [8, 2, 256, 1024]

    out = np.empty((B, S, D), np.float32)
    for i in range(NCORES):
        out[:, i * QBLK:(i + 1) * QBLK] = shards[i]
    return out
